# revision 69
# baseline (speedup 1.0000x reference)
"""ConvSwiGLU Trainium2 kernel: tensor-parallel over d_ff across 8 NeuronCores.

v5 design, 375us HW (vs 403us v2 baseline). Profile history: v2 408us span
(ACT 90%/DVE 93% busy); v3 537us (dual-scalar tensor_scalar loses fast DVE
modes; merged 3D tensor_tensor pays ~150-220cyc/row); v3.1 426us (PE idled
78us on evac backlogs + head + tail); v3.2 420us (chunk-granular conv
doubled DVE op count; microbench shows DVE per-op fixed overhead ~92ns TT
/ ~160ns TS regardless of width, so DVE wants FEW, WIDE ops); v4 390us
(center-tap elimination + interleaved chains); v5 375us (PE-diag drain +
split head/tail DMAs + p-state warmup).

  - All matmuls bf16 (fp8 DoubleRow is 2x but needs 3-term error
    compensation at this tolerance -> 1.5x bf16 cost; dead end).
  - Center-tap elimination: all taps of each side are scaled by 1/w2 on
    the host so the center tap is exactly 1.0 and the add tree reads the
    h slab DIRECTLY for it (no center premult op at all). The gate side
    is rescaled for free inside Silu's affine (scale=w2g, bias=tb2g); the
    up side's w2u is folded into Wd's rows on the host, and its bias
    tb2u/w2u rides the j3u ACT tap. Scaling by a constant keeps bf16
    RELATIVE precision, so numerics are unchanged (w2 clamped at 1e-6).
  - Conv at chunk-PAIR width (1024): premult tiles [128, GRP, 1028];
    DVE: j0/j4 @4x + j1 @2x_2p (24 TS/pair) + per-group add tree
    (32 TT/pair) + swiglu mult (4 TT/pair) ~= 18.2us/chunk.
    ACT: j3 both sides (8 taps/pair, shift-insensitive, up one carries
    the folded bias) + per-group Silu + 16 pure-copy [128,2,512] 2-bank
    psum evacuations ~= 16.3us/chunk. PE 20.4us/chunk paces.
  - Slab halo columns hold -b so pad taps contribute exactly 0.
  - produce/down PE chains emitted half-interleaved (pc1 pc2 dc1 dc2 pc3
    pc4 dc3 dc4) so a produce's 3rd chain never waits on an evac queued
    behind a conv tap batch (ps_main/ps_dn are only 2 tiles deep).
  - Pipeline: slot i = produce(i) halves + down(i-3) halves + conv piece
    (pair p = chunks 2p,2p+1: premults+taps+gate adds at slot 2p+2, up
    adds+silu+mult at slot 2p+3). x chunk DMA split 4 ways for the head
    (3 queues for chunks 0-1); chunk output DMA split across 2 queues.
  - Drain: the final pair's GATE conv runs on the then-idle PE as 5
    shifted diagonal matmuls accumulating in psum (diag(w_j/w2g) const),
    Silu reads psum directly, DVE only does the up side; down(13) (no dep
    on pair 7) fills PE between diag chains; the last two chunks' output
    DMAs go per-msub-pair on alternating queues. 16 dummy matmuls warm
    the PE p-state during the head x DMA.
  - Down matmul: psum[m,t] = sum_f Wd'[f,m] hact_s[f,t]; per-core partial
    yT summed on the host (bf16 partials, f32 host accumulate).
"""

import os
import sys
from contextlib import ExitStack

import ml_dtypes
import numpy as np

for _p in ("/root/.axon_site/_ro/trn_rl_repo", "/opt/trn_rl_repo"):
    if os.path.isdir(_p) and _p not in sys.path:
        sys.path.append(_p)

import concourse.bass as bass
import concourse.tile as tile
from concourse import bacc, mybir
from concourse.bass_utils import run_bass_kernel_spmd

F32 = mybir.dt.float32
BF16 = mybir.dt.bfloat16
AF = mybir.ActivationFunctionType
ALU = mybir.AluOpType

B, L, D = 4, 2048, 1024
F = 4096
NCORES = 8
FS = F // NCORES          # 512 d_ff channels per core
KSUB = D // 128           # 8 contraction subtiles for gate/up
GRP = FS // 128           # 4 channel groups per core
MSUB = D // 128           # 8 output row subtiles for down matmul
T = 512                   # token chunk (psum bank limit for f32)
CPS = L // T              # 4 chunks per sequence
NCH = (B * L) // T        # 16 chunks
NSEQ = B                  # 4 sequences
K = 5                     # conv taps
SLAB_W = L + 6            # 2 halo + 2048 tokens + 4 halo/pad
PW = 1024                 # conv pair width (2 chunks)
PRW = PW + 4              # premult tile width

_cache = {}


def _build_program():
    nc = bacc.Bacc("TRN2", target_bir_lowering=False, debug=False,
                   enable_asserts=False, num_devices=NCORES)

    xTc = nc.dram_tensor("xTc", [NCH, 128, KSUB, T], BF16, kind="ExternalInput").ap()
    wg = nc.dram_tensor("wgS", [128, KSUB, FS], BF16, kind="ExternalInput").ap()
    wu = nc.dram_tensor("wuS", [128, KSUB, FS], BF16, kind="ExternalInput").ap()
    wd = nc.dram_tensor("wdS", [128, GRP, D], BF16, kind="ExternalInput").ap()
    tsc = nc.dram_tensor("tscS", [128, 2, K, GRP], F32, kind="ExternalInput").ap()
    tb2 = nc.dram_tensor("tb2S", [128, 2, GRP], F32, kind="ExternalInput").ap()
    sw2 = nc.dram_tensor("sw2S", [128, GRP], F32, kind="ExternalInput").ap()
    nbh = nc.dram_tensor("nbhS", [128, 2, GRP, 6], BF16, kind="ExternalInput").ap()
    dgw = nc.dram_tensor("dgwS", [128, GRP, K, 128], BF16, kind="ExternalInput").ap()
    yT = nc.dram_tensor("yT", [D, B * L], BF16, kind="ExternalOutput").ap()

    with tile.TileContext(nc) as tc, ExitStack() as ctx:
        consts = ctx.enter_context(tc.tile_pool(name="consts", bufs=1))
        xpool = ctx.enter_context(tc.tile_pool(name="x", bufs=3))
        pp1 = ctx.enter_context(tc.tile_pool(name="pp1", bufs=1))
        ab1 = ctx.enter_context(tc.tile_pool(name="ab1", bufs=2))
        ab2 = ctx.enter_context(tc.tile_pool(name="ab2", bufs=2))
        hapool = ctx.enter_context(tc.tile_pool(name="ha", bufs=2))
        outpool = ctx.enter_context(tc.tile_pool(name="out", bufs=2))
        ps_main = ctx.enter_context(tc.tile_pool(name="psm", bufs=2, space="PSUM"))
        ps_dn = ctx.enter_context(tc.tile_pool(name="psd", bufs=2, space="PSUM"))

        # resident weights / constants
        wg_sb = consts.tile([128, KSUB, FS], BF16)
        wu_sb = consts.tile([128, KSUB, FS], BF16)
        wd_sb = consts.tile([128, GRP, D], BF16)
        tsc_sb = consts.tile([128, 2, K, GRP], F32)
        tb2_sb = consts.tile([128, 2, GRP], F32)
        sw2_sb = consts.tile([128, GRP], F32)
        dgw_sb = consts.tile([128, GRP, K, 128], BF16)
        # const loads on the Activation DMA queue (x/out use the SP queue);
        # wg first 2 ksubs first so chunk 0 matmuls can start immediately
        # head-critical loads (x chunk 0, wg, wu = 3MB) round-robin across
        # all three DMA queues in PE demand order, 256KB pieces: each queue
        # moves ~97GB/s, so any 1MB tensor on one queue lands ~3-8us after
        # the first chains need it
        xt0 = xpool.tile([128, KSUB, T], BF16, tag="xt")
        head_pieces = []
        for k in range(0, KSUB, 2):
            head_pieces.append((xt0[:, k:k + 2], xTc[0, :, k:k + 2]))
            head_pieces.append((wg_sb[:, k:k + 2], wg[:, k:k + 2]))
        for k in range(0, KSUB, 2):
            head_pieces.append((wu_sb[:, k:k + 2], wu[:, k:k + 2]))
        for j, (dst, src) in enumerate(head_pieces):
            (nc.scalar, nc.sync, nc.gpsimd)[j % 3].dma_start(dst, src)

        def load_late_consts():
            # emitted after produce(0): not needed until slot 2+
            nc.scalar.dma_start(wd_sb[:], wd)
            nc.scalar.dma_start(tsc_sb[:], tsc)
            nc.scalar.dma_start(tb2_sb[:], tb2)
            nc.scalar.dma_start(sw2_sb[:], sw2)
            nc.gpsimd.dma_start(slab[:, :, :, 0:2], nbh[:, :, :, 0:2])
            nc.gpsimd.dma_start(slab[:, :, :, 2 + L:SLAB_W],
                                nbh[:, :, :, 2:6])
            nc.scalar.dma_start(dgw_sb[:], dgw)

        # one shared h slab (sides x groups x padded seq); halo columns hold
        # -b so pure-mult premults contribute exactly 0 at pad positions.
        # Evacuations only ever touch the interior [2, 2+L). The halo DMAs
        # are deferred (load_late_consts) to keep gpsimd's queue free for
        # chunk 0's x pieces; conv(0) only needs them at slot 2.
        slab = consts.tile([128, 2, GRP, SLAB_W], BF16)

        ha_tiles = {}

        # PE p-state warmup: dummy matmuls with no DMA deps fill the head
        # DMA wait so real matmuls start at full clock (ramp needs ~3us of
        # continuous PE activity)
        warm_w = consts.tile([128, 128], BF16)
        # NB: this memset queues on gpsimd behind the head x/wu pieces, so
        # the warmup matmuls start ~13.6us in -- measured FASTER than an
        # early warmup, because they then fill the head piece-arrival gaps
        nc.gpsimd.memset(warm_w[:], 0.0)
        warm_ps = ps_dn.tile([128, 2, T], F32, tag="dn")
        for _ in range(22):
            nc.tensor.matmul(warm_ps[:, 0, 0:128], warm_w[:], warm_w[:],
                             start=True, stop=True)

        def produce_half(i, part):
            """gate (part 0) or up (part 1) matmuls for chunk i."""
            c = i % CPS
            if part == 0:
                if i == 0:
                    xt = xt0  # preloaded by the head round-robin
                else:
                    xt = xpool.tile([128, KSUB, T], BF16, tag="xt")
                    for n, k0 in enumerate(range(0, KSUB, 2)):
                        eng = nc.gpsimd if (i == 1 and n == 1) else nc.sync
                        eng.dma_start(xt[:, k0:k0 + 2], xTc[i, :, k0:k0 + 2])
                produce_half.xt = xt
            xt = produce_half.xt
            sd, w_sb = ((0, wg_sb), (1, wu_sb))[part]
            for gp in range(2):
                ps = ps_main.tile([128, 2, T], F32, tag="mm1")
                # NB: interleaving the two accumulation groups per-matmul
                # (alternating banks) measured FAR worse (447us) -- keep
                # the groups sequential
                for half in range(2):
                    g = gp * 2 + half
                    for ks in range(KSUB):
                        nc.tensor.matmul(
                            ps[:, half, :],
                            w_sb[:, ks, g * 128:(g + 1) * 128],
                            xt[:, ks, :],
                            start=(ks == 0), stop=(ks == KSUB - 1))
                nc.scalar.activation(
                    slab[:, sd, 2 * gp:2 * gp + 2, 2 + c * T:2 + (c + 1) * T],
                    ps[:], AF.Identity)

        def conv_a(p, last=False):
            """pair p (chunks 2p,2p+1): premults + taps + gate add tree.
            For the final pair (drain), ACT is otherwise idle so it takes
            ALL up-side taps, shortening the serial DVE chain."""
            t0 = (p % 2) * PW
            P = {}
            for sd in (0, 1):
                for j in (0, 1, 3, 4):
                    pj = pp1.tile([128, GRP, PRW], BF16, tag=f"p{sd}_{j}")
                    P[sd, j] = pj
                    dlt = j % 2
                    for g in range(GRP):
                        src = slab[:, sd, g, t0 + dlt:t0 + dlt + PRW]
                        sc = tsc_sb[:, sd, j, g:g + 1]
                        if j == 3 or (last and sd == 1):
                            # ACT: 1x, shift-insensitive; up side carries the
                            # folded (scaled) bias on j3
                            bias = (tb2_sb[:, 1, g:g + 1]
                                    if sd and j == 3 else 0.0)
                            nc.scalar.activation(pj[:, g, :], src, AF.Identity,
                                                 bias=bias, scale=sc)
                        else:
                            # DVE: j0/j4 at 4x (aligned), j1 at 2x_2p
                            nc.vector.tensor_scalar(pj[:, g, :], src, sc, None,
                                                    ALU.mult)
            conv_a.P = P
            _tree(P, 0, t0)

        def _tree(P, sd, t0):
            """per-group add tree; center tap reads the slab directly
            (coefficient 1.0 after the 1/w2 host scaling). tc aliases into
            p0's row and r into p4's row (both already consumed by ta)."""
            p0, p1, p3, p4 = (P[sd, j] for j in (0, 1, 3, 4))
            for g in range(GRP):
                ta = ab1.tile([128, PW], BF16, tag="ta")
                tb = ab1.tile([128, PW], BF16, tag="tb")
                nc.vector.tensor_tensor(ta[:], p0[:, g, 0:PW],
                                        p4[:, g, 4:4 + PW], ALU.add)
                nc.vector.tensor_tensor(tb[:], p1[:, g, 0:PW],
                                        p3[:, g, 2:2 + PW], ALU.add)
                nc.vector.tensor_tensor(p0[:, g, 0:PW], ta[:], tb[:], ALU.add)
                nc.vector.tensor_tensor(p4[:, g, 0:PW], p0[:, g, 0:PW],
                                        slab[:, sd, g, t0 + 2:t0 + 2 + PW],
                                        ALU.add)

        def conv_b(p):
            """pair p: up add tree + silu (rescales gate) + swiglu mult.
            (A per-group adds/silu/mult interleave measured no better --
            the residual ~1.2us/seq down-chain wait on the mult is DVE
            throughput in that window, not emission order.)"""
            t0 = (p % 2) * PW
            P = conv_a.P
            _tree(P, 1, t0)
            ha_t = hapool.tile([128, GRP, PW], BF16, tag="ha")
            for g in range(GRP):
                gact = ab2.tile([128, PW], BF16, tag="gact")
                nc.scalar.activation(gact[:], P[0, 4][:, g, 0:PW], AF.Silu,
                                     bias=tb2_sb[:, 0, g:g + 1],
                                     scale=sw2_sb[:, g:g + 1])
                nc.vector.tensor_tensor(ha_t[:, g, :], gact[:],
                                        P[1, 4][:, g, 0:PW], ALU.mult)
            ha_tiles[p] = ha_t

        yTr = yT.rearrange("(ms p) t -> p ms t", p=128)

        def down_half(i, part, tail=False):
            """down matmul pair-chains 2*part..2*part+1 for chunk i."""
            ha_t = ha_tiles[i // 2]
            off = (i % 2) * T
            if part == 0:
                down_half.out = outpool.tile([128, MSUB, T], BF16, tag="out")
            out_sb = down_half.out
            for mp in (2 * part, 2 * part + 1):
                dps = ps_dn.tile([128, 2, T], F32, tag="dn")
                for half in range(2):
                    ms = mp * 2 + half
                    for g in range(GRP):
                        nc.tensor.matmul(
                            dps[:, half, :],
                            wd_sb[:, g, ms * 128:(ms + 1) * 128],
                            ha_t[:, g, off:off + T],
                            start=(g == 0), stop=(g == GRP - 1))
                nc.scalar.activation(out_sb[:, 2 * mp:2 * mp + 2, :], dps[:],
                                     AF.Identity)
                if tail:
                    # drain: DMA each msub pair as soon as it's evacuated,
                    # alternating queues, so the final 1MB isn't serialized
                    # on one queue after the last matmul
                    eng = (nc.gpsimd, nc.sync)[mp % 2]
                    eng.dma_start(
                        yTr[:, 2 * mp:2 * mp + 2, i * T:(i + 1) * T],
                        out_sb[:, 2 * mp:2 * mp + 2, :])
            if part == 1 and not tail:
                # split the 1MB chunk output across two DMA queues; the
                # second half rides the scalar queue (same engine that just
                # wrote out_sb, and sync is ~74% loaded with x input, which
                # delayed out-tile recycling -> ACT evac WAR stalls)
                nc.gpsimd.dma_start(yTr[:, 0:4, i * T:(i + 1) * T],
                                    out_sb[:, 0:4, :])
                nc.scalar.dma_start(yTr[:, 4:8, i * T:(i + 1) * T],
                                    out_sb[:, 4:8, :])

        # slot i: produce(i) + down(i-3), chains half-interleaved; conv
        # pieces: pair p gets conv_a at slot 2p+2, conv_b at slot 2p+3
        # (emitted before down(2p), which consumes its ha)
        for i in range(NCH):
            d = i - 3
            if i % 2 == 0:
                produce_half(i, 0)
                if i == 0:
                    load_late_consts()
                if d >= 0:
                    down_half(d, 0)
                produce_half(i, 1)
                if d >= 0:
                    down_half(d, 1)
                if i >= 2:
                    p = i // 2 - 1
                    # even pairs: conv_a here (they need this slot's evacs
                    # for the halo). Odd (seq-final) pairs ran conv_a a
                    # slot early, so only conv_b lands here.
                    if p % 2 == 0:
                        conv_a(p)
                    else:
                        conv_b(p)
            else:
                produce_half(i, 0)
                q = i // 2 - 1
                if i >= 3 and q % 2 == 0:
                    conv_b(q)
                if d >= 0:
                    down_half(d, 0)
                produce_half(i, 1)
                if d >= 0:
                    down_half(d, 1)
                r = (i - 1) // 2
                if r % 2 == 1 and r < NCH // 2 - 1:
                    # seq-final pair: no next-chunk halo needed (tail is
                    # the -b columns), so its conv starts right after its
                    # own second chunk is produced
                    conv_a(r)
        # tail drain: the final pair's conv would serialize ~36us on DVE
        # after the last produce. Instead the GATE side runs on the now-idle
        # PE as 5 shifted diagonal matmuls accumulating in psum (diag(w_j)
        # loaded per group), Silu reads psum directly, and DVE only does
        # the up side. down(13) has no dep on pair 7 and fills PE between
        # diag chains.
        t0 = PW
        P = {}
        for j in (0, 1, 3, 4):
            pj = pp1.tile([128, GRP, PRW], BF16, tag=f"p1_{j}")
            P[1, j] = pj
            dlt = j % 2
            for g in range(GRP):
                src = slab[:, 1, g, t0 + dlt:t0 + dlt + PRW]
                sc = tsc_sb[:, 1, j, g:g + 1]
                if j == 3:
                    nc.scalar.activation(pj[:, g, :], src, AF.Identity,
                                         bias=tb2_sb[:, 1, g:g + 1], scale=sc)
                else:
                    nc.vector.tensor_scalar(pj[:, g, :], src, sc, None,
                                            ALU.mult)
        # gate group 3 stays on DVE/ACT (premults + adds + SBUF silu): PE
        # paces the tail at ~29us vs DVE ~20, so shifting one group's diag
        # work (~2.1us PE) to the less-loaded engines rebalances the drain
        gp3 = {}
        for j in (0, 1, 3, 4):
            pj3 = pp1.tile([128, GRP, PRW], BF16, tag=f"p0_{j}")
            gp3[j] = pj3
            dlt = j % 2
            src = slab[:, 0, 3, t0 + dlt:t0 + dlt + PRW]
            sc = tsc_sb[:, 0, j, 3:4]
            if j == 3:
                nc.scalar.activation(pj3[:, 3, :], src, AF.Identity, scale=sc)
            else:
                nc.vector.tensor_scalar(pj3[:, 3, :], src, sc, None, ALU.mult)
        gacts = {}
        for g in range(3):
            gps = ps_main.tile([128, 2, T], F32, tag="mm1")
            for half in range(2):
                for j in range(K):
                    nc.tensor.matmul(
                        gps[:, half, :], dgw_sb[:, g, j, :],
                        slab[:, 0, g, t0 + j + half * T:t0 + j + (half + 1) * T],
                        start=(j == 0), stop=(j == K - 1))
            gact = pp1.tile([128, 2, T], BF16, tag=f"gq{g}")
            nc.scalar.activation(gact[:], gps[:], AF.Silu,
                                 bias=tb2_sb[:, 0, g:g + 1],
                                 scale=sw2_sb[:, g:g + 1])
            gacts[g] = gact
            if g == 1:
                down_half(NCH - 3, 0)
        down_half(NCH - 3, 1)
        _tree(P, 1, t0)
        # group-3 gate add tree + silu from SBUF
        ta = ab1.tile([128, PW], BF16, tag="ta")
        tb = ab1.tile([128, PW], BF16, tag="tb")
        nc.vector.tensor_tensor(ta[:], gp3[0][:, 3, 0:PW],
                                gp3[4][:, 3, 4:4 + PW], ALU.add)
        nc.vector.tensor_tensor(tb[:], gp3[1][:, 3, 0:PW],
                                gp3[3][:, 3, 2:2 + PW], ALU.add)
        nc.vector.tensor_tensor(gp3[0][:, 3, 0:PW], ta[:], tb[:], ALU.add)
        nc.vector.tensor_tensor(gp3[4][:, 3, 0:PW], gp3[0][:, 3, 0:PW],
                                slab[:, 0, 3, t0 + 2:t0 + 2 + PW], ALU.add)
        gact3 = pp1.tile([128, 2, T], BF16, tag="gq3")
        nc.scalar.activation(gact3[:, 0, :], gp3[4][:, 3, 0:T], AF.Silu,
                             bias=tb2_sb[:, 0, 3:4], scale=sw2_sb[:, 3:4])
        nc.scalar.activation(gact3[:, 1, :], gp3[4][:, 3, T:PW], AF.Silu,
                             bias=tb2_sb[:, 0, 3:4], scale=sw2_sb[:, 3:4])
        gacts[3] = gact3
        ha_t = hapool.tile([128, GRP, PW], BF16, tag="ha")
        ha_tiles[NCH // 2 - 1] = ha_t
        for q in (0, 1):
            off = q * T
            for g in range(GRP):
                nc.vector.tensor_tensor(ha_t[:, g, off:off + T],
                                        gacts[g][:, q, :],
                                        P[1, 4][:, g, off:off + T], ALU.mult)
            down_half(NCH - 2 + q, 0, tail=True)
            down_half(NCH - 2 + q, 1, tail=True)

    nc.compile()
    return nc


def _prep_inputs(x, Wg, bgv, Wu, buv, convg_w, convg_b, convu_w, convu_b, Wd):
    """Host-side shard/layout. Returns list of per-core in_maps."""
    bf16 = ml_dtypes.bfloat16
    x = np.ascontiguousarray(x, np.float32)
    # [B, L, D] -> [B, KSUB, 128, L] -> chunks [NCH, 128, KSUB, T]
    xt = x.transpose(0, 2, 1).reshape(B, KSUB, 128, L)
    xTc = np.stack([
        xt[i // CPS, :, :, (i % CPS) * T:(i % CPS + 1) * T].transpose(1, 0, 2)
        for i in range(NCH)
    ]).astype(bf16)

    def colsplit(w, c):      # [D, F] -> per-core [128, KSUB, FS]
        s = np.asarray(w, np.float32)[:, c * FS:(c + 1) * FS]
        return np.ascontiguousarray(
            s.reshape(KSUB, 128, FS).transpose(1, 0, 2)).astype(bf16)

    def vecsplit(v, c):      # [F] -> [128, GRP] f32
        return np.ascontiguousarray(
            np.asarray(v, np.float32)[c * FS:(c + 1) * FS].reshape(GRP, 128).T)

    def tapsplit(w, c):      # [F, 1, K] -> [128, GRP, K] f32
        return np.ascontiguousarray(
            np.asarray(w, np.float32)[c * FS:(c + 1) * FS, 0, :]
            .reshape(GRP, 128, K).transpose(1, 0, 2))

    def clamp(w):            # keep sign, floor magnitude at 1e-6
        return np.where(np.abs(w) < 1e-6, np.where(w < 0, -1e-6, 1e-6), w)

    in_maps = []
    for c in range(NCORES):
        # per-side scaling: taps / w2 (center tap == 1.0, read directly);
        # gate rescaled inside silu (scale=w2g, bias=tb2g); up w2u folded
        # into Wd rows, up bias (scaled) rides the j3u tap
        tscS = np.zeros((128, 2, K, GRP), np.float32)
        tb2S = np.zeros((128, 2, GRP), np.float32)
        nbhS = np.zeros((128, 2, GRP, 6), np.float32)
        w2 = {}
        for sd, (cw, bv, cb) in enumerate(((convg_w, bgv, convg_b),
                                           (convu_w, buv, convu_b))):
            taps = tapsplit(cw, c)            # [128, GRP, K]
            bias = vecsplit(bv, c)            # [128, GRP]
            w2[sd] = clamp(taps[:, :, 2])
            tscS[:, sd] = (taps / w2[sd][:, :, None]).transpose(0, 2, 1)
            tb2S[:, sd] = bias * taps.sum(axis=2) + vecsplit(cb, c)
            nbhS[:, sd] = -bias[:, :, None]
        tb2S[:, 1] /= w2[1]                   # up bias lives in scaled domain
        sw2S = w2[0]                          # silu scale rescales the gate
        wdS = np.asarray(Wd, np.float32)[c * FS:(c + 1) * FS, :]
        wdS = wdS * w2[1].T.reshape(FS, 1)    # fold w2u into Wd rows
        dgwS = np.zeros((128, GRP, K, 128), np.float32)
        idx = np.arange(128)
        dgwS[idx, :, :, idx] = tscS[:, 0].transpose(0, 2, 1)
        in_maps.append({
            "xTc": xTc,
            "wgS": colsplit(Wg, c),
            "wuS": colsplit(Wu, c),
            "wdS": np.ascontiguousarray(
                wdS.reshape(GRP, 128, D).transpose(1, 0, 2)).astype(bf16),
            "tscS": tscS,
            "tb2S": tb2S,
            "sw2S": sw2S,
            "nbhS": nbhS.astype(bf16),
            "dgwS": dgwS.astype(bf16),
        })
    return in_maps


def run_on_cores(in_maps, **kwargs):
    if "nc" not in _cache:
        _cache["nc"] = _build_program()
    return run_bass_kernel_spmd(_cache["nc"], in_maps,
                                core_ids=list(range(NCORES)), **kwargs)


def kernel(x, Wg, bg, Wu, bu, convg_w, convg_b, convu_w, convu_b, Wd, bd):
    in_maps = _prep_inputs(x, Wg, bg, Wu, bu, convg_w, convg_b,
                           convu_w, convu_b, Wd)
    res = run_on_cores(in_maps)
    acc = np.zeros((D, B * L), np.float32)
    for r in res.results:
        acc += np.asarray(r["yT"], np.float32)
    acc += np.asarray(bd, np.float32)[:, None]
    return np.ascontiguousarray(acc.T.reshape(B, L, D)).astype(np.float32)


# revision 70
# speedup vs baseline: 1.0164x; 1.0164x over previous
"""ConvSwiGLU Trainium2 kernel: tensor-parallel over d_ff across 8 NeuronCores.

v5 design, 375us HW (vs 403us v2 baseline). Profile history: v2 408us span
(ACT 90%/DVE 93% busy); v3 537us (dual-scalar tensor_scalar loses fast DVE
modes; merged 3D tensor_tensor pays ~150-220cyc/row); v3.1 426us (PE idled
78us on evac backlogs + head + tail); v3.2 420us (chunk-granular conv
doubled DVE op count; microbench shows DVE per-op fixed overhead ~92ns TT
/ ~160ns TS regardless of width, so DVE wants FEW, WIDE ops); v4 390us
(center-tap elimination + interleaved chains); v5 375us (PE-diag drain +
split head/tail DMAs + p-state warmup).

  - All matmuls bf16 (fp8 DoubleRow is 2x but needs 3-term error
    compensation at this tolerance -> 1.5x bf16 cost; dead end).
  - Center-tap elimination: all taps of each side are scaled by 1/w2 on
    the host so the center tap is exactly 1.0 and the add tree reads the
    h slab DIRECTLY for it (no center premult op at all). The gate side
    is rescaled for free inside Silu's affine (scale=w2g, bias=tb2g); the
    up side's w2u is folded into Wd's rows on the host, and its bias
    tb2u/w2u rides the j3u ACT tap. Scaling by a constant keeps bf16
    RELATIVE precision, so numerics are unchanged (w2 clamped at 1e-6).
  - Conv at chunk-PAIR width (1024): premult tiles [128, GRP, 1028];
    DVE: j0/j4 @4x + j1 @2x_2p (24 TS/pair) + per-group add tree
    (32 TT/pair) + swiglu mult (4 TT/pair) ~= 18.2us/chunk.
    ACT: j3 both sides (8 taps/pair, shift-insensitive, up one carries
    the folded bias) + per-group Silu + 16 pure-copy [128,2,512] 2-bank
    psum evacuations ~= 16.3us/chunk. PE 20.4us/chunk paces.
  - Slab halo columns hold -b so pad taps contribute exactly 0.
  - produce/down PE chains emitted half-interleaved (pc1 pc2 dc1 dc2 pc3
    pc4 dc3 dc4) so a produce's 3rd chain never waits on an evac queued
    behind a conv tap batch (ps_main/ps_dn are only 2 tiles deep).
  - Pipeline: slot i = produce(i) halves + down(i-3) halves + conv piece
    (pair p = chunks 2p,2p+1: premults+taps+gate adds at slot 2p+2, up
    adds+silu+mult at slot 2p+3). x chunk DMA split 4 ways for the head
    (3 queues for chunks 0-1); chunk output DMA split across 2 queues.
  - Drain: the final pair's GATE conv runs on the then-idle PE as 5
    shifted diagonal matmuls accumulating in psum (diag(w_j/w2g) const),
    Silu reads psum directly, DVE only does the up side; down(13) (no dep
    on pair 7) fills PE between diag chains; the last two chunks' output
    DMAs go per-msub-pair on alternating queues. 16 dummy matmuls warm
    the PE p-state during the head x DMA.
  - Down matmul: psum[m,t] = sum_f Wd'[f,m] hact_s[f,t]; per-core partial
    yT summed on the host (bf16 partials, f32 host accumulate).
"""

import os
import sys
from contextlib import ExitStack

import ml_dtypes
import numpy as np

for _p in ("/root/.axon_site/_ro/trn_rl_repo", "/opt/trn_rl_repo"):
    if os.path.isdir(_p) and _p not in sys.path:
        sys.path.append(_p)

import concourse.bass as bass
import concourse.tile as tile
from concourse import bacc, mybir
from concourse.bass_utils import run_bass_kernel_spmd

F32 = mybir.dt.float32
BF16 = mybir.dt.bfloat16
AF = mybir.ActivationFunctionType
ALU = mybir.AluOpType

B, L, D = 4, 2048, 1024
F = 4096
NCORES = 8
FS = F // NCORES          # 512 d_ff channels per core
KSUB = D // 128           # 8 contraction subtiles for gate/up
GRP = FS // 128           # 4 channel groups per core
MSUB = D // 128           # 8 output row subtiles for down matmul
T = 512                   # token chunk (psum bank limit for f32)
CPS = L // T              # 4 chunks per sequence
NCH = (B * L) // T        # 16 chunks
NSEQ = B                  # 4 sequences
K = 5                     # conv taps
SLAB_W = L + 6            # 2 halo + 2048 tokens + 4 halo/pad
PW = 1024                 # conv pair width (2 chunks)
PRW = PW + 4              # premult tile width

_cache = {}


def _build_program():
    nc = bacc.Bacc("TRN2", target_bir_lowering=False, debug=False,
                   enable_asserts=False, num_devices=NCORES)

    xTc = nc.dram_tensor("xTc", [NCH, 128, KSUB, T], BF16, kind="ExternalInput").ap()
    wg = nc.dram_tensor("wgS", [128, KSUB, FS], BF16, kind="ExternalInput").ap()
    wu = nc.dram_tensor("wuS", [128, KSUB, FS], BF16, kind="ExternalInput").ap()
    wd = nc.dram_tensor("wdS", [128, GRP, D], BF16, kind="ExternalInput").ap()
    tsc = nc.dram_tensor("tscS", [128, 2, K, GRP], F32, kind="ExternalInput").ap()
    tb2 = nc.dram_tensor("tb2S", [128, 2, GRP], F32, kind="ExternalInput").ap()
    sw2 = nc.dram_tensor("sw2S", [128, GRP], F32, kind="ExternalInput").ap()
    nbh = nc.dram_tensor("nbhS", [128, 2, GRP, 6], BF16, kind="ExternalInput").ap()
    dgw = nc.dram_tensor("dgwS", [128, GRP, K, 128], BF16, kind="ExternalInput").ap()
    yT = nc.dram_tensor("yT", [D, B * L], BF16, kind="ExternalOutput").ap()

    with tile.TileContext(nc) as tc, ExitStack() as ctx:
        consts = ctx.enter_context(tc.tile_pool(name="consts", bufs=1))
        xpool = ctx.enter_context(tc.tile_pool(name="x", bufs=3))
        pp1 = ctx.enter_context(tc.tile_pool(name="pp1", bufs=1))
        ab1 = ctx.enter_context(tc.tile_pool(name="ab1", bufs=2))
        ab2 = ctx.enter_context(tc.tile_pool(name="ab2", bufs=2))
        hapool = ctx.enter_context(tc.tile_pool(name="ha", bufs=2))
        outpool = ctx.enter_context(tc.tile_pool(name="out", bufs=2))
        ps_main = ctx.enter_context(tc.tile_pool(name="psm", bufs=2, space="PSUM"))
        ps_dn = ctx.enter_context(tc.tile_pool(name="psd", bufs=2, space="PSUM"))

        # resident weights / constants
        wg_sb = consts.tile([128, KSUB, FS], BF16)
        wu_sb = consts.tile([128, KSUB, FS], BF16)
        wd_sb = consts.tile([128, GRP, D], BF16)
        tsc_sb = consts.tile([128, 2, K, GRP], F32)
        tb2_sb = consts.tile([128, 2, GRP], F32)
        sw2_sb = consts.tile([128, GRP], F32)
        dgw_sb = consts.tile([128, GRP, K, 128], BF16)
        # const loads on the Activation DMA queue (x/out use the SP queue);
        # wg first 2 ksubs first so chunk 0 matmuls can start immediately
        # head-critical loads (x chunk 0, wg, wu = 3MB) round-robin across
        # all three DMA queues in PE demand order, 256KB pieces: each queue
        # moves ~97GB/s, so any 1MB tensor on one queue lands ~3-8us after
        # the first chains need it
        xt0 = xpool.tile([128, KSUB, T], BF16, tag="xt")
        head_pieces = []
        for k in range(0, KSUB, 2):
            head_pieces.append((xt0[:, k:k + 2], xTc[0, :, k:k + 2]))
            head_pieces.append((wg_sb[:, k:k + 2], wg[:, k:k + 2]))
        for k in range(0, KSUB, 2):
            head_pieces.append((wu_sb[:, k:k + 2], wu[:, k:k + 2]))
        for j, (dst, src) in enumerate(head_pieces):
            (nc.scalar, nc.sync, nc.gpsimd)[j % 3].dma_start(dst, src)

        def load_late_consts():
            # emitted after produce(0): not needed until slot 2+
            nc.scalar.dma_start(wd_sb[:], wd)
            nc.scalar.dma_start(tsc_sb[:], tsc)
            nc.scalar.dma_start(tb2_sb[:], tb2)
            nc.scalar.dma_start(sw2_sb[:], sw2)
            nc.gpsimd.dma_start(slab[:, :, :, 0:2], nbh[:, :, :, 0:2])
            nc.gpsimd.dma_start(slab[:, :, :, 2 + L:SLAB_W],
                                nbh[:, :, :, 2:6])
            nc.scalar.dma_start(dgw_sb[:], dgw)

        # one shared h slab (sides x groups x padded seq); halo columns hold
        # -b so pure-mult premults contribute exactly 0 at pad positions.
        # Evacuations only ever touch the interior [2, 2+L). The halo DMAs
        # are deferred (load_late_consts) to keep gpsimd's queue free for
        # chunk 0's x pieces; conv(0) only needs them at slot 2.
        slab = consts.tile([128, 2, GRP, SLAB_W], BF16)

        ha_tiles = {}

        # PE p-state warmup: dummy matmuls with no DMA deps fill the head
        # DMA wait so real matmuls start at full clock (ramp needs ~3us of
        # continuous PE activity)
        warm_w = consts.tile([128, 128], BF16)
        # NB: this memset queues on gpsimd behind the head x/wu pieces, so
        # the warmup matmuls start ~13.6us in -- measured FASTER than an
        # early warmup, because they then fill the head piece-arrival gaps
        nc.gpsimd.memset(warm_w[:], 0.0)
        warm_ps = ps_dn.tile([128, 2, T], F32, tag="dn")
        for _ in range(16):
            nc.tensor.matmul(warm_ps[:, 0, 0:128], warm_w[:], warm_w[:],
                             start=True, stop=True)

        def produce_half(i, part):
            """gate (part 0) or up (part 1) matmuls for chunk i."""
            c = i % CPS
            if part == 0:
                if i == 0:
                    xt = xt0  # preloaded by the head round-robin
                else:
                    xt = xpool.tile([128, KSUB, T], BF16, tag="xt")
                    for n, k0 in enumerate(range(0, KSUB, 2)):
                        eng = nc.gpsimd if (i == 1 and n == 1) else nc.sync
                        eng.dma_start(xt[:, k0:k0 + 2], xTc[i, :, k0:k0 + 2])
                produce_half.xt = xt
            xt = produce_half.xt
            sd, w_sb = ((0, wg_sb), (1, wu_sb))[part]
            for gp in range(2):
                ps = ps_main.tile([128, 2, T], F32, tag="mm1")
                # NB: interleaving the two accumulation groups per-matmul
                # (alternating banks) measured FAR worse (447us) -- keep
                # the groups sequential
                for half in range(2):
                    g = gp * 2 + half
                    for ks in range(KSUB):
                        nc.tensor.matmul(
                            ps[:, half, :],
                            w_sb[:, ks, g * 128:(g + 1) * 128],
                            xt[:, ks, :],
                            start=(ks == 0), stop=(ks == KSUB - 1))
                nc.scalar.activation(
                    slab[:, sd, 2 * gp:2 * gp + 2, 2 + c * T:2 + (c + 1) * T],
                    ps[:], AF.Identity)

        def conv_a(p, last=False):
            """pair p (chunks 2p,2p+1): premults + taps + gate add tree.
            For the final pair (drain), ACT is otherwise idle so it takes
            ALL up-side taps, shortening the serial DVE chain."""
            t0 = (p % 2) * PW
            P = {}
            for sd in (0, 1):
                for j in (0, 1, 3, 4):
                    pj = pp1.tile([128, GRP, PRW], BF16, tag=f"p{sd}_{j}")
                    P[sd, j] = pj
                    dlt = j % 2
                    for g in range(GRP):
                        src = slab[:, sd, g, t0 + dlt:t0 + dlt + PRW]
                        sc = tsc_sb[:, sd, j, g:g + 1]
                        if j == 3 or (last and sd == 1):
                            # ACT: 1x, shift-insensitive; up side carries the
                            # folded (scaled) bias on j3
                            bias = (tb2_sb[:, 1, g:g + 1]
                                    if sd and j == 3 else 0.0)
                            nc.scalar.activation(pj[:, g, :], src, AF.Identity,
                                                 bias=bias, scale=sc)
                        else:
                            # DVE: j0/j4 at 4x (aligned), j1 at 2x_2p
                            nc.vector.tensor_scalar(pj[:, g, :], src, sc, None,
                                                    ALU.mult)
            conv_a.P = P
            _tree(P, 0, t0)

        def _tree(P, sd, t0):
            """per-group add tree; center tap reads the slab directly
            (coefficient 1.0 after the 1/w2 host scaling). tc aliases into
            p0's row and r into p4's row (both already consumed by ta)."""
            p0, p1, p3, p4 = (P[sd, j] for j in (0, 1, 3, 4))
            for g in range(GRP):
                ta = ab1.tile([128, PW], BF16, tag="ta")
                tb = ab1.tile([128, PW], BF16, tag="tb")
                nc.vector.tensor_tensor(ta[:], p0[:, g, 0:PW],
                                        p4[:, g, 4:4 + PW], ALU.add)
                nc.vector.tensor_tensor(tb[:], p1[:, g, 0:PW],
                                        p3[:, g, 2:2 + PW], ALU.add)
                nc.vector.tensor_tensor(p0[:, g, 0:PW], ta[:], tb[:], ALU.add)
                nc.vector.tensor_tensor(p4[:, g, 0:PW], p0[:, g, 0:PW],
                                        slab[:, sd, g, t0 + 2:t0 + 2 + PW],
                                        ALU.add)

        def conv_b(p):
            """pair p: up add tree + silu (rescales gate) + swiglu mult.
            (A per-group adds/silu/mult interleave measured no better --
            the residual ~1.2us/seq down-chain wait on the mult is DVE
            throughput in that window, not emission order.)"""
            t0 = (p % 2) * PW
            P = conv_a.P
            _tree(P, 1, t0)
            ha_t = hapool.tile([128, GRP, PW], BF16, tag="ha")
            for g in range(GRP):
                gact = ab2.tile([128, PW], BF16, tag="gact")
                nc.scalar.activation(gact[:], P[0, 4][:, g, 0:PW], AF.Silu,
                                     bias=tb2_sb[:, 0, g:g + 1],
                                     scale=sw2_sb[:, g:g + 1])
                nc.vector.tensor_tensor(ha_t[:, g, :], gact[:],
                                        P[1, 4][:, g, 0:PW], ALU.mult)
            ha_tiles[p] = ha_t

        yTr = yT.rearrange("(ms p) t -> p ms t", p=128)

        def down_half(i, part, tail=False):
            """down matmul pair-chains 2*part..2*part+1 for chunk i."""
            ha_t = ha_tiles[i // 2]
            off = (i % 2) * T
            if part == 0:
                down_half.out = outpool.tile([128, MSUB, T], BF16, tag="out")
            out_sb = down_half.out
            for mp in (2 * part, 2 * part + 1):
                dps = ps_dn.tile([128, 2, T], F32, tag="dn")
                for half in range(2):
                    ms = mp * 2 + half
                    for g in range(GRP):
                        nc.tensor.matmul(
                            dps[:, half, :],
                            wd_sb[:, g, ms * 128:(ms + 1) * 128],
                            ha_t[:, g, off:off + T],
                            start=(g == 0), stop=(g == GRP - 1))
                nc.scalar.activation(out_sb[:, 2 * mp:2 * mp + 2, :], dps[:],
                                     AF.Identity)
                if tail:
                    # drain: DMA each msub pair as soon as it's evacuated,
                    # alternating queues, so the final 1MB isn't serialized
                    # on one queue after the last matmul
                    eng = (nc.gpsimd, nc.sync)[mp % 2]
                    eng.dma_start(
                        yTr[:, 2 * mp:2 * mp + 2, i * T:(i + 1) * T],
                        out_sb[:, 2 * mp:2 * mp + 2, :])
            if part == 1 and not tail:
                # split the 1MB chunk output across two DMA queues; the
                # second half rides the scalar queue (same engine that just
                # wrote out_sb, and sync is ~74% loaded with x input, which
                # delayed out-tile recycling -> ACT evac WAR stalls)
                nc.gpsimd.dma_start(yTr[:, 0:4, i * T:(i + 1) * T],
                                    out_sb[:, 0:4, :])
                nc.scalar.dma_start(yTr[:, 4:8, i * T:(i + 1) * T],
                                    out_sb[:, 4:8, :])

        # slot i: produce(i) + down(i-3), chains half-interleaved; conv
        # pieces: pair p gets conv_a at slot 2p+2, conv_b at slot 2p+3
        # (emitted before down(2p), which consumes its ha)
        for i in range(NCH):
            d = i - 3
            if i % 2 == 0:
                produce_half(i, 0)
                if i == 0:
                    load_late_consts()
                if d >= 0:
                    down_half(d, 0)
                produce_half(i, 1)
                if d >= 0:
                    down_half(d, 1)
                if i >= 2:
                    p = i // 2 - 1
                    # even pairs: conv_a here (they need this slot's evacs
                    # for the halo). Odd (seq-final) pairs ran conv_a a
                    # slot early, so only conv_b lands here.
                    if p % 2 == 0:
                        conv_a(p)
                    else:
                        conv_b(p)
            else:
                produce_half(i, 0)
                q = i // 2 - 1
                if i >= 3 and q % 2 == 0:
                    conv_b(q)
                if d >= 0:
                    down_half(d, 0)
                produce_half(i, 1)
                if d >= 0:
                    down_half(d, 1)
                r = (i - 1) // 2
                if r % 2 == 1 and r < NCH // 2 - 1:
                    # seq-final pair: no next-chunk halo needed (tail is
                    # the -b columns), so its conv starts right after its
                    # own second chunk is produced
                    conv_a(r)
        # tail drain: the final pair's conv would serialize ~36us on DVE
        # after the last produce. Instead the GATE side runs on the now-idle
        # PE as 5 shifted diagonal matmuls accumulating in psum (diag(w_j)
        # loaded per group), Silu reads psum directly, and DVE only does
        # the up side. down(13) has no dep on pair 7 and fills PE between
        # diag chains.
        t0 = PW
        P = {}
        for j in (0, 1, 3, 4):
            pj = pp1.tile([128, GRP, PRW], BF16, tag=f"p1_{j}")
            P[1, j] = pj
            dlt = j % 2
            for g in range(GRP):
                src = slab[:, 1, g, t0 + dlt:t0 + dlt + PRW]
                sc = tsc_sb[:, 1, j, g:g + 1]
                if j == 3:
                    nc.scalar.activation(pj[:, g, :], src, AF.Identity,
                                         bias=tb2_sb[:, 1, g:g + 1], scale=sc)
                else:
                    nc.vector.tensor_scalar(pj[:, g, :], src, sc, None,
                                            ALU.mult)
        # gate group 3 stays on DVE/ACT (premults + adds + SBUF silu): PE
        # paces the tail at ~29us vs DVE ~20, so shifting one group's diag
        # work (~2.1us PE) to the less-loaded engines rebalances the drain
        gp3 = {}
        for j in (0, 1, 3, 4):
            pj3 = pp1.tile([128, GRP, PRW], BF16, tag=f"p0_{j}")
            gp3[j] = pj3
            dlt = j % 2
            src = slab[:, 0, 3, t0 + dlt:t0 + dlt + PRW]
            sc = tsc_sb[:, 0, j, 3:4]
            if j == 3:
                nc.scalar.activation(pj3[:, 3, :], src, AF.Identity, scale=sc)
            else:
                nc.vector.tensor_scalar(pj3[:, 3, :], src, sc, None, ALU.mult)
        gacts = {}
        for g in range(3):
            gps = ps_main.tile([128, 2, T], F32, tag="mm1")
            for half in range(2):
                for j in range(K):
                    nc.tensor.matmul(
                        gps[:, half, :], dgw_sb[:, g, j, :],
                        slab[:, 0, g, t0 + j + half * T:t0 + j + (half + 1) * T],
                        start=(j == 0), stop=(j == K - 1))
            gact = pp1.tile([128, 2, T], BF16, tag=f"gq{g}")
            nc.scalar.activation(gact[:], gps[:], AF.Silu,
                                 bias=tb2_sb[:, 0, g:g + 1],
                                 scale=sw2_sb[:, g:g + 1])
            gacts[g] = gact
            if g == 1:
                down_half(NCH - 3, 0)
        down_half(NCH - 3, 1)
        _tree(P, 1, t0)
        # group-3 gate add tree + silu from SBUF
        ta = ab1.tile([128, PW], BF16, tag="ta")
        tb = ab1.tile([128, PW], BF16, tag="tb")
        nc.vector.tensor_tensor(ta[:], gp3[0][:, 3, 0:PW],
                                gp3[4][:, 3, 4:4 + PW], ALU.add)
        nc.vector.tensor_tensor(tb[:], gp3[1][:, 3, 0:PW],
                                gp3[3][:, 3, 2:2 + PW], ALU.add)
        nc.vector.tensor_tensor(gp3[0][:, 3, 0:PW], ta[:], tb[:], ALU.add)
        nc.vector.tensor_tensor(gp3[4][:, 3, 0:PW], gp3[0][:, 3, 0:PW],
                                slab[:, 0, 3, t0 + 2:t0 + 2 + PW], ALU.add)
        gact3 = pp1.tile([128, 2, T], BF16, tag="gq3")
        nc.scalar.activation(gact3[:, 0, :], gp3[4][:, 3, 0:T], AF.Silu,
                             bias=tb2_sb[:, 0, 3:4], scale=sw2_sb[:, 3:4])
        nc.scalar.activation(gact3[:, 1, :], gp3[4][:, 3, T:PW], AF.Silu,
                             bias=tb2_sb[:, 0, 3:4], scale=sw2_sb[:, 3:4])
        gacts[3] = gact3
        ha_t = hapool.tile([128, GRP, PW], BF16, tag="ha")
        ha_tiles[NCH // 2 - 1] = ha_t
        for q in (0, 1):
            off = q * T
            for g in range(GRP):
                nc.vector.tensor_tensor(ha_t[:, g, off:off + T],
                                        gacts[g][:, q, :],
                                        P[1, 4][:, g, off:off + T], ALU.mult)
            down_half(NCH - 2 + q, 0, tail=True)
            down_half(NCH - 2 + q, 1, tail=True)

    nc.compile()
    return nc


def _prep_inputs(x, Wg, bgv, Wu, buv, convg_w, convg_b, convu_w, convu_b, Wd):
    """Host-side shard/layout. Returns list of per-core in_maps."""
    bf16 = ml_dtypes.bfloat16
    x = np.ascontiguousarray(x, np.float32)
    # [B, L, D] -> [B, KSUB, 128, L] -> chunks [NCH, 128, KSUB, T]
    xt = x.transpose(0, 2, 1).reshape(B, KSUB, 128, L)
    xTc = np.stack([
        xt[i // CPS, :, :, (i % CPS) * T:(i % CPS + 1) * T].transpose(1, 0, 2)
        for i in range(NCH)
    ]).astype(bf16)

    def colsplit(w, c):      # [D, F] -> per-core [128, KSUB, FS]
        s = np.asarray(w, np.float32)[:, c * FS:(c + 1) * FS]
        return np.ascontiguousarray(
            s.reshape(KSUB, 128, FS).transpose(1, 0, 2)).astype(bf16)

    def vecsplit(v, c):      # [F] -> [128, GRP] f32
        return np.ascontiguousarray(
            np.asarray(v, np.float32)[c * FS:(c + 1) * FS].reshape(GRP, 128).T)

    def tapsplit(w, c):      # [F, 1, K] -> [128, GRP, K] f32
        return np.ascontiguousarray(
            np.asarray(w, np.float32)[c * FS:(c + 1) * FS, 0, :]
            .reshape(GRP, 128, K).transpose(1, 0, 2))

    def clamp(w):            # keep sign, floor magnitude at 1e-6
        return np.where(np.abs(w) < 1e-6, np.where(w < 0, -1e-6, 1e-6), w)

    in_maps = []
    for c in range(NCORES):
        # per-side scaling: taps / w2 (center tap == 1.0, read directly);
        # gate rescaled inside silu (scale=w2g, bias=tb2g); up w2u folded
        # into Wd rows, up bias (scaled) rides the j3u tap
        tscS = np.zeros((128, 2, K, GRP), np.float32)
        tb2S = np.zeros((128, 2, GRP), np.float32)
        nbhS = np.zeros((128, 2, GRP, 6), np.float32)
        w2 = {}
        for sd, (cw, bv, cb) in enumerate(((convg_w, bgv, convg_b),
                                           (convu_w, buv, convu_b))):
            taps = tapsplit(cw, c)            # [128, GRP, K]
            bias = vecsplit(bv, c)            # [128, GRP]
            w2[sd] = clamp(taps[:, :, 2])
            tscS[:, sd] = (taps / w2[sd][:, :, None]).transpose(0, 2, 1)
            tb2S[:, sd] = bias * taps.sum(axis=2) + vecsplit(cb, c)
            nbhS[:, sd] = -bias[:, :, None]
        tb2S[:, 1] /= w2[1]                   # up bias lives in scaled domain
        sw2S = w2[0]                          # silu scale rescales the gate
        wdS = np.asarray(Wd, np.float32)[c * FS:(c + 1) * FS, :]
        wdS = wdS * w2[1].T.reshape(FS, 1)    # fold w2u into Wd rows
        dgwS = np.zeros((128, GRP, K, 128), np.float32)
        idx = np.arange(128)
        dgwS[idx, :, :, idx] = tscS[:, 0].transpose(0, 2, 1)
        in_maps.append({
            "xTc": xTc,
            "wgS": colsplit(Wg, c),
            "wuS": colsplit(Wu, c),
            "wdS": np.ascontiguousarray(
                wdS.reshape(GRP, 128, D).transpose(1, 0, 2)).astype(bf16),
            "tscS": tscS,
            "tb2S": tb2S,
            "sw2S": sw2S,
            "nbhS": nbhS.astype(bf16),
            "dgwS": dgwS.astype(bf16),
        })
    return in_maps


def run_on_cores(in_maps, **kwargs):
    if "nc" not in _cache:
        _cache["nc"] = _build_program()
    return run_bass_kernel_spmd(_cache["nc"], in_maps,
                                core_ids=list(range(NCORES)), **kwargs)


def kernel(x, Wg, bg, Wu, bu, convg_w, convg_b, convu_w, convu_b, Wd, bd):
    in_maps = _prep_inputs(x, Wg, bg, Wu, bu, convg_w, convg_b,
                           convu_w, convu_b, Wd)
    res = run_on_cores(in_maps)
    acc = np.zeros((D, B * L), np.float32)
    for r in res.results:
        acc += np.asarray(r["yT"], np.float32)
    acc += np.asarray(bd, np.float32)[:, None]
    return np.ascontiguousarray(acc.T.reshape(B, L, D)).astype(np.float32)
